# revision 42
# baseline (speedup 1.0000x reference)
"""DiscreteOptionActor Trainium2 kernel.

Computes, for each sample b, logits = MLP_{option[b]}(obs[b]) where each of
the 16 options has its own 3-layer MLP (128 -> 256 -> 256 -> 18, ReLU).

Strategy (MoE routing, expert-sharded, all fp16 on device):
  - Host groups samples by option (argsort). Each core gets two option slots
    (slot 0: one of the 8 smallest groups, slot 1: one of the 8 largest so the
    kernel ends on slot 1's short tail pair), each padded to a runtime-sized
    per-slot pad (roundup-128 of the slot max, capped with host fallback),
    stored transposed (feature-major [128, pad]) in fp16.
  - Weights packed into two byte tensors per option (w1+b1 gates L1; w2+w3+b2
    follows), one DMA + one semaphore each. Time-critical DMAs (w1b0, early
    xt0 chunks) ride the sync HWDGE queue; the scalar queue starts ~1.3us
    late (ACT table load) and gpsimd SWDGE has ~4us latency, so those carry
    late-need transfers only.
  - Gapless warmup matmuls from program entry ramp the HAM clock gate
    (1.2 -> 2.4 GHz) while the first transfers land, sized so the clock goes
    warm right as the first data arrives (~10us into the NEFF).
  - L1/L2 run through a 3-slot PSUM rotation; each [128,1024] fill is drained
    by a single [128,1024] bias+ReLU instruction, alternating ScalarE and
    VectorE per fill (halves the per-instruction fixed overhead and the
    semaphore traffic vs split 512-col drains).
  - L3 (M=18, K=256) is emitted once per TWO pairs, k-major across four
    512-col blocks mapped to the four PE column-group quadrants of one
    dedicated PSUM tile (4-way concurrent matmuls via tile_position, ~0.5
    passes/col), drained as [128,512] copies (garbage rows included) and
    DMA'd out packed; the host unpacks. The L3 groups also pad the rotation
    slot-reuse latency so the PE never waits on drains.
  - Trailing dummy matmuls keep the PE HAM-warm through the fixed walrus
    semaphore-reset epilogue (the PE sequencer's epilogue resets run ~2x
    faster at K=8/8), and cover the last output DMAs.
  - Host scatters results back to original row order and adds b3.
"""

import numpy as np

B, OBS, OPT, H1, H2, A = 65536, 128, 16, 256, 256, 18
NCORES = 8
OPC = OPT // NCORES  # options per core = 2
PAD_CAP = 4096  # beyond this, overflow rows are computed on host

_CACHE = {}


def _mk_pairs(pad):
    out = []
    st = 0
    while st < pad:
        nb = min(1024, pad - st)
        out.append((st, nb))
        st += nb
    return out


def _mk_chunks(pad):
    # 512-col lead chunks for an early L1 start, then ~1024s (each extra
    # early DMA costs ~1.3us of cold-queue re-arm, so few, biggish chunks)
    sizes = [512, 512, 1024]
    out = []
    st = 0
    for s in sizes:
        if st >= pad:
            break
        nb = min(s, pad - st)
        out.append((st, nb))
        st += nb
    if st < pad:
        rem = pad - st
        if rem > 1536:
            first = (rem // 2 + 127) // 128 * 128
            out.append((st, first))
            out.append((st + first, rem - first))
        else:
            out.append((st, rem))
    return out


WARMUPS = [512] * 8 + [256]

# ALL weights ride at the head of each xt tensor so they arrive with the
# first x chunk: w1 fp16 [256 cols]; b1 f32 [2] as 4 fp16 cols; w2 fp16
# [2,256] as 512 cols; w3 fp16 [2,18] as 36 cols; b2 f32 [2] as 4 cols.
W1B_COLS = 812


def _halves(nb):
    out = []
    h = 0
    while h < nb:
        w = min(512, nb - h)
        out.append((h, w))
        h += w
    return out


def _chunk_hi(chunks, st, nb):
    """Index of the last chunk overlapping [st, st+nb)."""
    hi = 0
    for ci, (cst, cnb) in enumerate(chunks):
        if cst < st + nb:
            hi = ci
    return hi


def _build_v4(pads):
    import concourse.bass as bass
    import concourse.bacc as bacc
    import concourse.mybir as mybir

    f32 = mybir.dt.float32
    f16 = mybir.dt.float16
    AF = mybir.ActivationFunctionType
    ALU = mybir.AluOpType

    PAIRS_O = {o: _mk_pairs(pads[o]) for o in range(OPC)}
    XCHUNKS_O = {o: _mk_chunks(pads[o]) for o in range(OPC)}

    nc = bacc.Bacc(None, target_bir_lowering=False, debug=False)
    xt = [nc.declare_dram_parameter(f"xt{o}", [OBS, W1B_COLS + pads[o]], f16,
                                    isOutput=False)
          for o in range(OPC)]
    out = [nc.declare_dram_parameter(f"out{o}", [A, pads[o]], f16, isOutput=True)
           for o in range(OPC)]
    # packed L3 output: psum quadrant layout [128, 1024] per option; block b
    # (512 logits cols) sits at rows 32*(b%4)..+18, cols 512*(b//4)..+512
    outp = [nc.declare_dram_parameter(f"outp{o}", [128, 1024], f16, isOutput=True)
            for o in range(OPC)]

    # --- on-chip tensors ---
    xts = [nc.alloc_sbuf_tensor(f"xts{o}", [OBS, W1B_COLS + pads[o]], f16)
           for o in range(OPC)]
    h1s = [[nc.alloc_sbuf_tensor(f"h1_{o}_{c}", [128, pads[o]], f16) for c in range(2)]
           for o in range(OPC)]
    h2s = [[nc.alloc_sbuf_tensor(f"h2_{o}_{m}", [128, pads[o]], f16) for m in range(2)]
           for o in range(OPC)]
    osbs = [nc.alloc_sbuf_tensor(f"osb{o}", [A, pads[o]], f16) for o in range(OPC)]
    osbp = [nc.alloc_sbuf_tensor(f"osbp{o}", [128, 1024], f16) for o in range(OPC)]
    dummy = nc.alloc_sbuf_tensor("warm_dummy", [128, 512], f16)
    dummy_o = nc.alloc_sbuf_tensor("warm_dummy_o", [128, 1], f32)
    dma_wake = nc.alloc_sbuf_tensor("dma_wake", [128, 8], f16)

    pss = [nc.alloc_psum_tensor(f"ps{s}", [128, 1024], f32) for s in range(4)]

    def w1_ap(o, c):
        return xts[o].ap()[:, c * 128:(c + 1) * 128]

    def b1_ap(o, c):
        return xts[o].ap()[:, 256:260].bitcast(f32)[:, c:c + 1]

    def x_ap(o, st, w):
        return xts[o].ap()[:, W1B_COLS + st:W1B_COLS + st + w]

    def w2_ap(o, k, m):
        base = 260 + k * 256 + m * 128
        return xts[o].ap()[:, base:base + 128]

    def w3_ap(o, k):
        return xts[o].ap()[:, 772 + k * 18:772 + (k + 1) * 18]

    def b2_ap(o, m):
        return xts[o].ap()[:, 808:812].bitcast(f32)[:, m:m + 1]

    # --- semaphores ---
    xsem = [[nc.alloc_semaphore(f"x{o}_{ci}") for ci in range(len(XCHUNKS_O[o]))]
            for o in range(OPC)]
    fd = [nc.alloc_semaphore(f"fd{s}") for s in range(4)]
    prog = {}
    for o in range(OPC):
        for key in ("h1a", "h1v", "h2a", "h2v", "oa", "ov"):
            prog[(key, o)] = nc.alloc_semaphore(f"{key}{o}")
    odsem = nc.alloc_semaphore("od")
    wakesem = nc.alloc_semaphore("wake")

    # --- static schedule containers ---
    pe_ops = []
    act_ops = []
    dve_ops = []
    sync_ops = []
    scalar_ops = []
    scalar_tail_ops = []
    gps_ops = []

    fill_count = [0, 0, 0, 0]
    slot_prev_drain = [None, None, None, None]
    fill_idx = 0
    prog_count = {k: 0 for k in prog}
    od_count = [0]

    pe_last_wait = {}

    def pe_wait(waits, sem, val):
        key = sem.name if hasattr(sem, "name") else id(sem)
        if pe_last_wait.get(key, -1) < val:
            waits.append((sem, val))
            pe_last_wait[key] = val

    di = 0  # drain-engine round robin

    def emit_fill(data_waits, mms, out_part, drains):
        """mms: (h, w, lhs_fn, rhs_fn, start, stop, mm_waits, tile_pos, ps_lo)
        drains: list of (engine, kind, dst_fn, bias_fn, psem_key, src_lo,
        src_hi, h0, w0)
        """
        nonlocal fill_idx
        s = fill_idx % 4
        fill_idx += 1
        waits = []
        if slot_prev_drain[s] is not None:
            for sem, cnt in slot_prev_drain[s]:
                pe_wait(waits, sem, cnt)
        for sem, val in data_waits:
            pe_wait(waits, sem, val)
        pe_ops.append((waits, s, mms, out_part, fd[s]))
        fill_count[s] += 1
        fd_thresh = fill_count[s]
        newprev = []
        for (drain_engine, kind, dst_fn, bias_fn, psem_key, src_lo, src_hi,
             h0, w0) in drains:
            sem = prog[psem_key]
            prog_count[psem_key] += 1
            cnt = prog_count[psem_key]
            op = ([(fd[s], fd_thresh)], kind, s, dst_fn, bias_fn, sem,
                  src_lo, src_hi, h0, w0)
            if drain_engine == "act":
                act_ops.append(op)
            else:
                dve_ops.append(op)
            newprev.append((sem, cnt))
        slot_prev_drain[s] = newprev
        return {k: prog_count[k] for k in prog}

    # --- input DMA schedule ---
    def xdma(o, ci):
        # chunk 0 is extended to cover the W1B_COLS weight head, so the L1
        # weights arrive with the first x columns in a single transfer
        cst, cnb = XCHUNKS_O[o][ci]
        lo = 0 if ci == 0 else W1B_COLS + cst
        hi = W1B_COLS + cst + cnb
        return ("dma", [],
                (lambda o=o, lo=lo, hi=hi: xts[o].ap()[:, lo:hi]),
                (lambda o=o, lo=lo, hi=hi: xt[o][:, lo:hi]),
                xsem[o][ci], 16)

    # The 16 DMA engines are one shared pool that round-robins across
    # queues with pending work, and a cold engine pays ~1.5-2us to wake on
    # its first packet. A 256-byte wake transfer touching all 16 engines
    # goes first; the early-critical sequence then rides the sync queue
    # ALONE (full pool, strict need order). The gpsimd queue (which carries
    # nothing else) is gated behind the critical window; the scalar queue
    # carries NO input DMAs - a gated trigger there would stall ScalarE's
    # drain stream.
    # the wake rides the otherwise-DMA-free scalar queue, in parallel with
    # the sync queue's first real chunk
    scalar_ops.append(("dma", [],
                       (lambda: dma_wake.ap()[:]),
                       (lambda: xt[0][:, 0:8]), wakesem, 16))
    def chunk_need_order(o, pair_order):
        seen, order = set(), []
        for p in pair_order:
            st, nb = PAIRS_O[o][p]
            for ci, (cst, cnb) in enumerate(XCHUNKS_O[o]):
                if cst < st + nb and cst + cnb > st and ci not in seen:
                    seen.add(ci)
                    order.append(ci)
        return order

    def pair_order(o):
        full = [p for p, (st, nb) in enumerate(PAIRS_O[o])
                if nb == 1024 and (st + nb) <= 4096]
        keep = (len(full) // 2) * 2
        tail = sorted([p for p in range(len(PAIRS_O[o])) if p not in full]
                      + full[keep:])
        return tail + full[:keep]

    _o0 = chunk_need_order(0, pair_order(0))
    _o1 = chunk_need_order(1, pair_order(1))
    sync_ops.extend([xdma(0, ci) for ci in _o0])
    gate_g = [(xsem[0][min(2, len(XCHUNKS_O[0]) - 1)], 16)]
    for i, op in enumerate([xdma(1, ci) for ci in _o1]):
        gps_ops.append((op[0], gate_g if i == 0 else [], op[2], op[3],
                        op[4], op[5]))

    # measured per-drain engine costs (ns) incl. the ~130ns semaphore wait.
    # 512-col drains hit the 2x-accelerated mode on both engines (a 1024-col
    # fp32 PSUM read crosses the 2KB bank boundary and loses the accel), so
    # every drain is kept at <=512 columns.
    eng_load = {"act": 0.0, "dve": 0.0}

    def drain_eng(w):
        ta = (w + 300) / 1.2 + 130
        tv = (w + 190) / 0.96 + 130
        if eng_load["act"] + ta <= eng_load["dve"] + tv:
            eng_load["act"] += ta
            return "act"
        eng_load["dve"] += tv
        return "dve"

    l1_thr = {}
    l2_thr = {}

    def emit_l1_split(o, p):
        # first pair only: chunk-granular single-MM fills ordered h-major,
        # so the leading matmuls depend only on the first small xt chunks
        st, nb = PAIRS_O[o][p]
        pc = None
        spans = []
        for cst, cnb in XCHUNKS_O[o]:
            lo = max(cst, st)
            hi_ = min(cst + cnb, st + nb)
            if lo < hi_:
                h = lo
                while h < hi_:
                    w = min(512, hi_ - h)
                    spans.append((h - st, w))
                    h += w
        for h, w in spans:
            for c in range(2):
                hi = _chunk_hi(XCHUNKS_O[o], st + h, w)
                mms = [(
                    h, w,
                    (lambda o=o, c=c: w1_ap(o, c)),
                    (lambda o=o, st=st, h=h, w=w: x_ap(o, st + h, w)),
                    True, True, [(xsem[o][hi], 16)], None, 0,
                )]
                eng = drain_eng(w)
                drains = [(eng, "relu",
                           (lambda o=o, c=c, st=st, h=h, w=w: h1s[o][c].ap()[:, st + h:st + h + w]),
                           (lambda o=o, c=c: b1_ap(o, c)),
                           ("h1a" if eng == "act" else "h1v", o), 0, 128, h, w)]
                pc = emit_fill([(xsem[o][0], 16)], mms, 128, drains)
        l1_thr[(o, p)] = (pc[("h1a", o)], pc[("h1v", o)])

    def emit_l1(o, p):
        # one fill per c-chunk ([128, nb]), single full-width drain
        st, nb = PAIRS_O[o][p]
        pc = None
        for c in range(2):
            mms = []
            for h, w in _halves(nb):
                hi = _chunk_hi(XCHUNKS_O[o], st + h, w)
                mms.append((
                    h, w,
                    (lambda o=o, c=c: w1_ap(o, c)),
                    (lambda o=o, st=st, h=h, w=w: x_ap(o, st + h, w)),
                    True, True, [(xsem[o][hi], 16)], None, 0,
                ))
            eng = drain_eng(nb)
            drains = [(eng, "relu",
                       (lambda o=o, c=c, st=st, nb=nb: h1s[o][c].ap()[:, st:st + nb]),
                       (lambda o=o, c=c: b1_ap(o, c)),
                       ("h1a" if eng == "act" else "h1v", o), 0, 128, 0, nb)]
            pc = emit_fill([(xsem[o][0], 16)], mms, 128, drains)
        l1_thr[(o, p)] = (pc[("h1a", o)], pc[("h1v", o)])

    def emit_l2(o, p, split=False):
        st, nb = PAIRS_O[o][p]
        na, nv = l1_thr[(o, p)]
        pc = None
        for m in range(2):
            data_waits = [(xsem[o][0], 16)]
            if na:
                data_waits.append((prog[("h1a", o)], na))
            if nv:
                data_waits.append((prog[("h1v", o)], nv))
            mms = []
            for h, w in _halves(nb):
                for k in range(2):
                    mms.append((
                        h, w,
                        (lambda o=o, k=k, m=m: w2_ap(o, k, m)),
                        (lambda o=o, k=k, st=st, h=h, w=w: h1s[o][k].ap()[:, st + h:st + h + w]),
                        k == 0, k == 1, None, None, 0,
                    ))
            if split and nb > 512:
                # final pair: halve drain latency so the trailing L3 group
                # is not stalled on a full-width drain
                drains = []
                for hh, ww in _halves(nb):
                    eng = drain_eng(ww)
                    drains.append((eng, "relu",
                                   (lambda o=o, m=m, st=st, hh=hh, ww=ww: h2s[o][m].ap()[:, st + hh:st + hh + ww]),
                                   (lambda o=o, m=m: b2_ap(o, m)),
                                   ("h2a" if eng == "act" else "h2v", o),
                                   0, 128, hh, ww))
            else:
                eng = drain_eng(nb)
                drains = [(eng, "relu",
                           (lambda o=o, m=m, st=st, nb=nb: h2s[o][m].ap()[:, st:st + nb]),
                           (lambda o=o, m=m: b2_ap(o, m)),
                           ("h2a" if eng == "act" else "h2v", o), 0, 128, 0, nb)]
            pc = emit_fill(data_waits, mms, 128, drains)
        l2_thr[(o, p)] = (pc[("h2a", o)], pc[("h2v", o)])

    def emit_l3_group(o, g, plist):
        """Packed path: 4 blocks (2048 cols, pairs plist) k-major into the 4
        PE column-group quadrants of one rotation slot; 4-way concurrent."""
        # the first pair's two blocks gate only on that pair's h2 drains
        # (long since complete); the second pair's drains gate its blocks at
        # the mm level, so the leading matmuls overlap the trailing drains
        na0, nv0 = l2_thr[(o, plist[0])]
        na1, nv1 = l2_thr[(o, plist[1])]
        data_waits = [(xsem[o][0], 16)]
        if na0:
            data_waits.append((prog[("h2a", o)], na0))
        if nv0:
            data_waits.append((prog[("h2v", o)], nv0))
        late = []
        if na1 > na0:
            late.append((prog[("h2a", o)], na1))
        if nv1 > nv0:
            late.append((prog[("h2v", o)], nv1))
        mms = []
        for bi, k in ((0, 0), (1, 0), (0, 1), (1, 1),
                      (2, 0), (3, 0), (2, 1), (3, 1)):
            b = 4 * g + bi
            q = b % 4
            mms.append((
                0, 512,
                (lambda o=o, k=k: w3_ap(o, k)),
                (lambda o=o, k=k, b=b: h2s[o][k].ap()[:, b * 512:(b + 1) * 512]),
                k == 0, k == 1, late if (bi, k) == (2, 0) else None,
                (0, 32 * q), 32 * q,
            ))
        c0 = g * 512
        eng = drain_eng(512)
        key = ("oa" if eng == "act" else "ov", o)
        drains = [(eng, "copy",
                   (lambda o=o, c0=c0: osbp[o].ap()[:, c0:c0 + 512]), None,
                   key, 0, 128, 0, 512)]
        pc = emit_fill(data_waits, mms, A, drains)
        od_count[0] += 1
        # the very last packed half rides the scalar queue so the final two
        # output DMAs issue in parallel across two queues
        q_ = scalar_tail_ops if (o == OPC - 1 and g == 1) else sync_ops
        q_.append(("dma", [(prog[key], pc[key])],
                   (lambda o=o, c0=c0: outp[o][:, c0:c0 + 512]),
                   (lambda o=o, c0=c0: osbp[o].ap()[:, c0:c0 + 512]),
                   odsem, 16))

    def emit_l3_tail(o, p):
        # tail path: normal rotation fill + [A, nb] drain + DMA
        st, nb = PAIRS_O[o][p]
        na, nv = l2_thr[(o, p)]
        data_waits = [(xsem[o][0], 16)]
        if na:
            data_waits.append((prog[("h2a", o)], na))
        if nv:
            data_waits.append((prog[("h2v", o)], nv))
        mms = []
        for h, w in _halves(nb):
            for k in range(2):
                mms.append((
                    h, w,
                    (lambda o=o, k=k: w3_ap(o, k)),
                    (lambda o=o, k=k, st=st, h=h, w=w: h2s[o][k].ap()[:, st + h:st + h + w]),
                    k == 0, k == 1, None, None, 0,
                ))
        eng = drain_eng(nb)
        key = ("oa" if eng == "act" else "ov", o)
        drains = [(eng, "copy",
                   (lambda o=o, st=st, nb=nb: osbs[o].ap()[:, st:st + nb]),
                   None,
                   key, 0, A, 0, nb)]
        pc = emit_fill(data_waits, mms, A, drains)
        od_count[0] += 1
        dma_op = ("dma", [(prog[key], pc[key])],
                  (lambda o=o, st=st, nb=nb: out[o][:, st:st + nb]),
                  (lambda o=o, st=st, nb=nb: osbs[o].ap()[:, st:st + nb]),
                  odsem, 16)
        sync_ops.append(dma_op)

    # global software pipeline: L1 two pair-groups ahead; L3 in groups of two
    # 1024-pairs (4 blocks), trailing L2 by one pair; sub-1024 tail pairs go
    # through the rotation path and are processed FIRST within their option
    # so the schedule ends on a packed L3 group, not a serialized tail chain.
    l3full = {o: [p for p, (st, nb) in enumerate(PAIRS_O[o])
                  if nb == 1024 and (st + nb) <= 4096]
              for o in range(OPC)}
    l3tail = {o: [p for p in range(len(PAIRS_O[o])) if p not in l3full[o]]
              for o in range(OPC)}

    # only complete 2-pair groups take the packed path; odd leftovers fall
    # back to the rotation tail path
    for o in range(OPC):
        keep = (len(l3full[o]) // 2) * 2
        l3tail[o] = sorted(l3tail[o] + l3full[o][keep:])
        l3full[o] = l3full[o][:keep]

    l1q = [(o, p) for o in range(OPC)
           for p in (l3tail[o] + l3full[o])]
    l2q = list(l1q)

    emit_l1_split(*l1q.pop(0))
    emit_l1(*l1q.pop(0))
    emit_l1(*l1q.pop(0))
    l3ready = []  # (o, p) with L2 done, awaiting grouping

    def try_emit_l3(exclude_last):
        pool = l3ready[:-1] if exclude_last else l3ready
        for (oo, pp) in list(pool):
            if pp in l3tail[oo]:
                emit_l3_tail(oo, pp)
                l3ready.remove((oo, pp))
        pool = l3ready[:-1] if exclude_last else l3ready
        for oo in range(OPC):
            fulls = [pp for (o2, pp) in pool if o2 == oo and pp in l3full[oo]]
            while len(fulls) >= 2:
                g = fulls[0] // 2
                emit_l3_group(oo, g, fulls[:2])
                for pp in fulls[:2]:
                    l3ready.remove((oo, pp))
                fulls = fulls[2:]

    for qi, (o, p) in enumerate(l2q):
        emit_l2(o, p, split=(qi == len(l2q) - 1))
        if l1q:
            emit_l1(*l1q.pop(0))
        l3ready.append((o, p))
        # emit L3 once a pair's L2 is at least one pair-group old
        try_emit_l3(exclude_last=True)
    try_emit_l3(exclude_last=False)


    # --- emit engine programs ---
    with nc.Block(no_gpsimd_drain=True) as block:

        @block.gpsimd
        def _(eng):
            for op in gps_ops:
                kind, waits, dst_fn, src_fn, sem, val = op
                for wsem_, wval in waits:
                    eng.wait_ge(wsem_, wval)
                eng.dma_start(out=dst_fn(), in_=src_fn()).then_inc(sem, val)

        @block.sync
        def _(eng):
            for op in sync_ops:
                kind, waits, dst_fn, src_fn, sem, val = op
                for wsem_, wval in waits:
                    eng.wait_ge(wsem_, wval)
                eng.dma_start(out=dst_fn(), in_=src_fn()).then_inc(sem, val)

        @block.tensor
        def _(eng):
            for wn in WARMUPS:
                nc.tensor.matmul(
                    pss[0].ap()[:128, :wn], dummy.ap()[:, :128], dummy.ap()[:, :wn],
                    start=True, stop=True,
                )
            mm_seen = {}
            for waits, s, mms, out_part, fdsem in pe_ops:
                for wsem_, wval in waits:
                    eng.wait_ge(wsem_, wval)
                for j, (h, w, lhs_fn, rhs_fn, stt, stp, mwaits, tp, plo) in enumerate(mms):
                    if mwaits:
                        for wsem_, wval in mwaits:
                            key = wsem_.name if hasattr(wsem_, "name") else id(wsem_)
                            if mm_seen.get(key, -1) < wval:
                                eng.wait_ge(wsem_, wval)
                                mm_seen[key] = wval
                    kw = {}
                    if tp is not None:
                        kw["tile_position"] = tp
                    inst = nc.tensor.matmul(
                        pss[s].ap()[plo:plo + out_part, h:h + w],
                        lhs_fn(), rhs_fn(), start=stt, stop=stp, **kw,
                    )
                    if j == len(mms) - 1:
                        inst.then_inc(fdsem, 1)

        @block.scalar
        def _(eng):
            for op in scalar_ops:
                kind, waits, dst_fn, src_fn, sem, val = op
                for wsem_, wval in waits:
                    eng.wait_ge(wsem_, wval)
                eng.dma_start(out=dst_fn(), in_=src_fn()).then_inc(sem, val)
            nc.scalar.activation(dummy_o.ap()[:], dummy.ap()[:, 0:1], AF.Relu, bias=0.0)
            for waits, kind, s, dst_fn, bias_fn, sem, lo, hi, h0, w0 in act_ops:
                for wsem_, wval in waits:
                    eng.wait_ge(wsem_, wval)
                dst = dst_fn()
                src = pss[s].ap()[lo:hi, h0:h0 + w0]
                if kind == "relu":
                    inst = nc.scalar.activation(dst, src, AF.Relu, bias=bias_fn())
                else:
                    inst = nc.scalar.activation(dst, src, AF.Copy)
                inst.then_inc(sem, 1)
            for op in scalar_tail_ops:
                kind, waits, dst_fn, src_fn, sem, val = op
                for wsem_, wval in waits:
                    eng.wait_ge(wsem_, wval)
                eng.dma_start(out=dst_fn(), in_=src_fn()).then_inc(sem, val)

        @block.vector
        def _(eng):
            for waits, kind, s, dst_fn, bias_fn, sem, lo, hi, h0, w0 in dve_ops:
                for wsem_, wval in waits:
                    eng.wait_ge(wsem_, wval)
                dst = dst_fn()
                src = pss[s].ap()[lo:hi, h0:h0 + w0]
                if kind == "relu":
                    inst = nc.vector.tensor_scalar(
                        dst, src, bias_fn(), 0.0, ALU.add, ALU.max
                    )
                else:
                    inst = nc.vector.tensor_copy(dst, src)
                inst.then_inc(sem, 1)

    nc.compile()
    return nc


def _get_program(pads):
    key = tuple(pads)
    if key not in _CACHE:
        _CACHE[key] = _build_v4(pads)
    return _CACHE[key]


def _prep(inputs):
    obs = np.ascontiguousarray(np.asarray(inputs["obs"], dtype=np.float32))
    option = np.asarray(inputs["option"]).astype(np.int64, copy=False)
    W1 = np.asarray(inputs["W1"], dtype=np.float32)
    b1 = np.asarray(inputs["b1"], dtype=np.float32)
    W2 = np.asarray(inputs["W2"], dtype=np.float32)
    b2 = np.asarray(inputs["b2"], dtype=np.float32)
    W3 = np.asarray(inputs["W3"], dtype=np.float32)
    b3 = np.asarray(inputs["b3"], dtype=np.float32)

    order = np.argsort(option, kind="stable")
    sorted_opt = option[order]
    starts = np.searchsorted(sorted_opt, np.arange(OPT + 1))
    idx_per_opt = [order[starts[o]: starts[o + 1]] for o in range(OPT)]
    counts = np.array([len(ix) for ix in idx_per_opt])

    # slot 0 (processed first) gets the 8 smallest groups, slot 1 the 8
    # largest: the kernel then ends on slot 1's short tail pair
    by_size = np.argsort(-counts, kind="stable")
    slot_opts = [list(by_size[NCORES:]), list(by_size[:NCORES])]
    pads = []
    for s in range(OPC):
        mx = max(counts[o] for o in slot_opts[s])
        pads.append(int(min(-(-max(mx, 128) // 128) * 128, PAD_CAP)))
    pads = tuple(pads)

    W1B_COLS = 812

    def pack_w(o):
        # [128, 812] fp16 head: w1 [256] ; b1 f32 [2] as 4 cols ; w2 [512] ;
        # w3 [36] ; b2 f32 [2] as 4 cols
        w1p = np.ascontiguousarray(W1[o].astype(np.float16))  # [128, 256]
        b1p = np.ascontiguousarray(b1[o].reshape(2, 128).T.astype(np.float32))
        w2p = np.ascontiguousarray(
            W2[o].reshape(2, 128, H2).transpose(1, 0, 2).astype(np.float16)
        ).reshape(128, -1)
        w3p = np.ascontiguousarray(
            W3[o].reshape(2, 128, A).transpose(1, 0, 2).astype(np.float16)
        ).reshape(128, -1)
        b2p = np.ascontiguousarray(b2[o].reshape(2, 128).T.astype(np.float32))
        return np.concatenate([w1p, b1p.view(np.float16), w2p,
                               w3p, b2p.view(np.float16)], axis=1)

    in_maps = []
    for core in range(NCORES):
        m = {}
        for s in range(OPC):
            o = slot_opts[s][core]
            idx = idx_per_opt[o][:pads[s]]
            xtc = np.zeros((OBS, W1B_COLS + pads[s]), np.float16)
            xtc[:, :W1B_COLS] = pack_w(o)
            xtc[:, W1B_COLS:W1B_COLS + len(idx)] = obs[idx].T
            m[f"xt{s}"] = xtc
        in_maps.append(m)
    host = dict(obs=obs, W1=W1, b1=b1, W2=W2, b2=b2, W3=W3, b3=b3)
    return in_maps, idx_per_opt, slot_opts, pads, host


def _unshard(results, idx_per_opt, slot_opts, pads, host):
    out_full = np.empty((B, 1, A), np.float32)
    for core in range(NCORES):
        for s in range(OPC):
            o = slot_opts[s][core]
            resp = results[core][f"outp{s}"]  # [128, 1024] packed quadrants
            rest = results[core][f"out{s}"]   # [A, pads[s]] tail only
            idx = idx_per_opt[o]
            n = min(len(idx), pads[s])
            full_span = min((pads[s] // 1024) * 1024, 4096)
            logits = np.empty((A, pads[s]), np.float32)
            for b in range(full_span // 512):
                q, ch = b % 4, b // 4
                logits[:, b * 512:(b + 1) * 512] = \
                    resp[32 * q:32 * q + A, ch * 512:(ch + 1) * 512]
            if full_span < pads[s]:
                logits[:, full_span:] = rest[:, full_span:]
            out_full[idx[:n], 0, :] = logits[:, :n].T + host["b3"][o]
            if len(idx) > n:  # overflow beyond pad: compute on host
                rows = host["obs"][idx[n:]]
                h = np.maximum(rows @ host["W1"][o] + host["b1"][o], 0.0)
                h = np.maximum(h @ host["W2"][o] + host["b2"][o], 0.0)
                out_full[idx[n:], 0, :] = h @ host["W3"][o] + host["b3"][o]
    return out_full


def run(inputs, trace=False, **spmd_kwargs):
    """Run the kernel; returns (output, BassKernelResults)."""
    from concourse.bass_utils import run_bass_kernel_spmd

    import time as _time

    in_maps, idx_per_opt, slot_opts, pads, host = _prep(inputs)
    br = None
    for attempt in range(3):
        try:
            nc = _get_program(pads)
            br = run_bass_kernel_spmd(
                nc, in_maps, list(range(NCORES)), trace=trace, **spmd_kwargs
            )
            break
        except Exception:
            # transient device/runtime hiccups: rebuild and retry
            _CACHE.clear()
            if attempt == 2:
                raise
            _time.sleep(2.0)
    return _unshard(br.results, idx_per_opt, slot_opts, pads, host), br


def kernel(**inputs):
    out, _ = run(inputs)
    return out
